# revision 31
# baseline (speedup 1.0000x reference)
"""Masked-attention kernel for 8 TRN2 NeuronCores (batch-parallel sharding).

Per-core shard: 2 batches of [S=2048, D=128] Q/K/V + [S, S] bool mask.
Layout strategy (per core):
  - scores are computed TRANSPOSED (S^T[k, q]) so the PV matmul consumes the
    exp() output directly with V in its natural [k, d] layout.
  - the mask is folded into the scores inside the PE accumulation: one fp8
    matmul per (k-tile, q-subtile) with the mask chunk (DMA-cast u8->fp8e4)
    as the stationary operand and a -240*I fp8 identity as the moving
    operand; exp() then flushes masked entries to ~0. Eight plain 128-col
    matmuls beat four DoubleRow ones on HW: with walrus ldw-opt disabled,
    per-matmul weight (re)loads dominate, and small stationaries pipeline.
  - softmax denominator: DVE accumulates exp tiles across k-tiles, then per
    q-subtile one [acc-chunk]^T @ ones matmul gives the denominator as a
    PSUM column; reciprocal on DVE; applied as a per-partition scalar after
    the final transpose.
  - PV lags TWO k-tiles so the PE never waits on exp() and the previous
    q-chunk's epilogue copies drain opsum before the start=True write.
  - each q-chunk's epilogue (denominator matmuls, PSUM->SBUF copies,
    O^T->O transposes, 1/den scaling, store) is deferred into the next
    q-chunk's first four k-slots so the in-order PE/DVE streams never
    head-of-line block on it; the last q-chunk flushes at the end.
  - batch 1's loads/casts/transposes are phased across fixed k-slots of
    q-chunks 1-2 (never more than the 4-deep engine wait queues can park).
  - maskp bufs=2 throttles the SWDGE mask stream: chunk N+2's DMA waits on
    chunk N's consumers, so mask transfers cannot flood the shared DMA
    device ahead of the latency-critical K/Q quarter loads at startup.
"""

import numpy as np
import ml_dtypes

B, S, D = 16, 2048, 128
NCORES = 8
BP = B // NCORES  # batches per core
P = 128
QC = 1024  # q-chunk (columns of the transposed score tile)
NQC = S // QC
NKT = S // P  # k tiles
NQS = QC // P  # q subtiles per chunk
MM_N = 512  # matmul moving free dim
SCALE = 1.0 / float(np.sqrt(128.0))
MASK_NEG = -240.0
MASK_DR = False  # DoubleRow mask matmuls (slower on HW, faster in CoreSim)
ACT_COPIES = False  # epilogue PSUM->SBUF copies on ACT instead of DVE
QK_FIRST = False  # QK matmuls before mask matmuls within each k-tile
PV_LAG = 2  # k-tiles of lag between exp() and the consuming PV matmul

_CACHE = {}


def build_nc(loop=True, mask_dr=None, act_copies=None, qk_first=None,
             pv_lag=None):
    global MASK_DR, ACT_COPIES, QK_FIRST, PV_LAG
    if mask_dr is not None:
        MASK_DR = mask_dr
    if act_copies is not None:
        ACT_COPIES = act_copies
    if qk_first is not None:
        QK_FIRST = qk_first
    if pv_lag is not None:
        PV_LAG = pv_lag
    import concourse.mybir as mybir
    import concourse.tile as tile
    from concourse import bacc

    fp16 = mybir.dt.float16
    fp32 = mybir.dt.float32

    nc = bacc.Bacc("TRN2", target_bir_lowering=False, debug=False,
                   num_devices=NCORES)

    Qd = nc.dram_tensor("Q", [BP, S, D], fp32, kind="ExternalInput")
    Kd = nc.dram_tensor("K", [BP, S, D], fp32, kind="ExternalInput")
    Vd = nc.dram_tensor("V", [BP, S, D], fp32, kind="ExternalInput")
    Md = nc.dram_tensor("mask", [BP, S, S], mybir.dt.uint8, kind="ExternalInput")
    if loop:
        # run-count knob for differential HW timing (graded path: loop=False)
        Id = nc.dram_tensor("iters", [1, 1], mybir.dt.int32,
                            kind="ExternalInput")
    Od = nc.dram_tensor("out", [BP, S, D], fp32, kind="ExternalOutput")

    # DoubleRow moving operand: out[k, n] = sum_{q,i} M[(sq0+i)*128+q, k]
    # * R[q, i, n]; R[q, i, n] = -240 iff n == i*128 + q (block-diag
    # identities), so one fp8 DoubleRow matmul folds TWO 128-q mask chunks
    # at 0.5 cycles/row.
    negI_np = np.zeros((P, 2, 2 * P), dtype=np.float32)
    for q in range(P):
        negI_np[q, 0, q] = MASK_NEG
        negI_np[q, 1, P + q] = MASK_NEG
    negI_np = negI_np.reshape(P, 2, 2 * P).astype(ml_dtypes.float8_e4m3)
    negI_dram = nc.inline_tensor(negI_np, name="negI_const")
    ident_dram = nc.inline_tensor(np.eye(P, dtype=np.float16),
                                  name="ident_const")

    with tile.TileContext(nc) as tc:
        with tc.tile_pool(name="consts", bufs=1) as consts, \
             tc.tile_pool(name="stag", bufs=3) as stag, \
             tc.tile_pool(name="qkv", bufs=1) as qkv, \
             tc.tile_pool(name="maskp", bufs=2) as maskp, \
             tc.tile_pool(name="pp", bufs=4) as pp, \
             tc.tile_pool(name="accp", bufs=2) as accp, \
             tc.tile_pool(name="outp", bufs=2) as outp, \
             tc.tile_pool(name="spsum", bufs=2, space="PSUM") as spsum, \
             tc.tile_pool(name="opsum", bufs=1, space="PSUM") as opsum, \
             tc.tile_pool(name="tpsum", bufs=2, space="PSUM") as tpsum:

            # consts ride separate trigger rings so neither delays Kq0/Qq0
            negI = consts.tile([P, 2, 2 * P], mybir.dt.float8e4)
            nc.sync.dma_start(out=negI[:, :, :], in_=negI_dram.ap())
            ident = consts.tile([P, P], fp16)
            nc.scalar.dma_start(out=ident[:, :], in_=ident_dram.ap())
            ones_col = consts.tile([P, 1], fp16)
            nc.vector.memset(ones_col, 1.0)

            pools = (stag, qkv, maskp, pp, accp, outp, spsum, opsum, tpsum)
            if loop:
                it_sb = consts.tile([1, 1], mybir.dt.int32)
                nc.sync.dma_start(out=it_sb[:, :], in_=Id.ap())
                n_iters = nc.values_load(it_sb[:, :],
                                         skip_runtime_bounds_check=True)
                with tc.For_i(0, n_iters, 1,
                              hint_engines=(mybir.EngineType.PE,
                                            mybir.EngineType.Activation,
                                            mybir.EngineType.DVE,
                                            mybir.EngineType.SP,
                                            mybir.EngineType.Pool)):
                    _kernel_body(nc, tc, mybir, Qd, Kd, Vd, Md, Od, negI,
                                 ident, ones_col, *pools)
            else:
                _kernel_body(nc, tc, mybir, Qd, Kd, Vd, Md, Od, negI,
                             ident, ones_col, *pools)
    nc.compile()
    return nc


def _kernel_body(nc, tc, mybir, Qd, Kd, Vd, Md, Od, negI, ident, ones_col,
                 stag, qkv, maskp, pp, accp, outp, spsum, opsum, tpsum):
    fp16 = mybir.dt.float16
    fp32 = mybir.dt.float32
    fp8 = mybir.dt.float8e4
    Exp = mybir.ActivationFunctionType.Exp

    MC = 512  # mask column-chunk (k) per DMA

    def load_mask_ck(b, qc, ck):
        # one tile per 512-column chunk: a single writer DMA, so the first
        # consuming matmul doesn't wait on later chunks (tile-granular deps).
        # maskp bufs=2 throttles the SWDGE stream: chunk N+2's DMA waits on
        # chunk N's consumers, so masks can't flood the shared DMA device.
        t = maskp.tile([P, NQS, MC], fp8, name="mfck")
        nc.gpsimd.dma_start(
            out=t[:, :, :],
            in_=Md.ap()[b, qc * QC:(qc + 1) * QC, ck * MC:(ck + 1) * MC]
                .rearrange("(s p) k -> p s k", p=P))
        return t

    # first mask chunk before everything else (SWDGE rides its own gen path)
    mf00 = [None] * (S // MC)
    mf00[0] = load_mask_ck(0, 0, 0)

    HT = NKT // 2  # tiles per half
    QT4 = HT // 2  # tiles per quarter

    ktt = {b: [qkv.tile([P, QT4 * P], fp16, name=f"ktt{b}{q}")
               for q in range(4)] for b in range(BP)}
    qt = {b: [qkv.tile([P, QT4 * P], fp16, name=f"qt{b}{q}")
              for q in range(4)] for b in range(BP)}
    vsb = {b: [qkv.tile([P, HT, D], fp16, name=f"vsb{b}{h}")
               for h in range(2)] for b in range(BP)}

    def load_f32_half(src_ap, b, h, ring, name="ldf", bufs=None):
        f = stag.tile([P, HT, D], fp32, name=name, bufs=bufs)
        ring(out=f[:, :, :],
             in_=src_ap[b, h * HT * P:(h + 1) * HT * P, :]
                 .rearrange("(t p) d -> p t d", p=P))
        return f

    def transpose_quarter(src_ap, dst, b, q4, ring):
        f = stag.tile([P, QT4, D], fp32, name="ldf4", bufs=4)
        ring(out=f[:, :, :],
             in_=src_ap[b, q4 * QT4 * P:(q4 + 1) * QT4 * P, :]
                 .rearrange("(t p) d -> p t d", p=P))
        g = stag.tile([P, QT4, D], fp16, name="ldh4")
        nc.vector.tensor_copy(out=g[:, :, :], in_=f[:, :, :])
        tps = tpsum.tile([P, QT4 * P], fp16, name="tps")
        for t in range(QT4):
            nc.tensor.transpose(tps[:, t * P:(t + 1) * P],
                                g[:, t, :], ident[:, :])
        nc.vector.tensor_copy(out=dst[:, :], in_=tps[:, :])

    def transpose_half(g, dstA, dstB):
        tps = tpsum.tile([P, HT * P], fp16, name="tps")
        for t in range(HT):
            nc.tensor.transpose(tps[:, t * P:(t + 1) * P],
                                g[:, t, :], ident[:, :])
        nc.vector.tensor_copy(out=dstA[:, :], in_=tps[:, :QT4 * P])
        nc.vector.tensor_copy(out=dstB[:, :], in_=tps[:, QT4 * P:])

    # ---- b0 prep: eager quarters; issue order = shared-DMA-gen priority:
    # the k-loop needs ktt[0][0], qt[0][0], qt[0][1], V h0 first.
    b0state = {}

    def prep_b0():
        transpose_quarter(Kd.ap(), ktt[0][0], 0, 0, nc.sync.dma_start)
        transpose_quarter(Qd.ap(), qt[0][0], 0, 0, nc.scalar.dma_start)
        transpose_quarter(Qd.ap(), qt[0][1], 0, 1, nc.scalar.dma_start)
        mf00[1] = load_mask_ck(0, 0, 1)
        transpose_quarter(Kd.ap(), ktt[0][1], 0, 1, nc.sync.dma_start)
        vf0 = load_f32_half(Vd.ap(), 0, 0, nc.sync.dma_start, "vf")
        nc.vector.tensor_copy(out=vsb[0][0][:, :, :], in_=vf0[:, :, :])
        mf00[2] = load_mask_ck(0, 0, 2)
        fk1 = load_f32_half(Kd.ap(), 0, 1, nc.sync.dma_start)
        mf00[3] = load_mask_ck(0, 0, 3)
        vf1 = load_f32_half(Vd.ap(), 0, 1, nc.sync.dma_start, "vf")
        nc.vector.tensor_copy(out=vsb[0][1][:, :, :], in_=vf1[:, :, :])
        fq1 = load_f32_half(Qd.ap(), 0, 1, nc.scalar.dma_start)
        b0state["f"] = (fk1, fq1)

    def finish_b0_cast():
        fk1, fq1 = b0state["f"]
        gk1 = stag.tile([P, HT, D], fp16, name="ldh")
        nc.vector.tensor_copy(out=gk1[:, :, :], in_=fk1[:, :, :])
        gq1 = stag.tile([P, HT, D], fp16, name="ldh")
        nc.vector.tensor_copy(out=gq1[:, :, :], in_=fq1[:, :, :])
        b0state["g"] = (gk1, gq1)

    def finish_b0_transpose():
        gk1, gq1 = b0state["g"]
        transpose_half(gk1, ktt[0][2], ktt[0][3])
        transpose_half(gq1, qt[0][2], qt[0][3])

    prep_b0()

    # ---- b1 prep: loads/casts/transposes phased into gi=1 and gi=2 k-slots
    # so no engine ever parks >wait-queue depth on not-yet-loaded data.
    b1state = {}

    def b1_loads():
        ctx = tc.tile_wait_until(0.020)
        ctx.__enter__()
        b1state["fk0"] = load_f32_half(Kd.ap(), 1, 0, nc.sync.dma_start,
                                       "pfk0", 1)
        b1state["fq0"] = load_f32_half(Qd.ap(), 1, 0, nc.scalar.dma_start,
                                       "pfq0", 1)
        b1state["fv0"] = load_f32_half(Vd.ap(), 1, 0, nc.sync.dma_start,
                                       "pfv0", 1)
        b1state["fv1"] = load_f32_half(Vd.ap(), 1, 1, nc.sync.dma_start,
                                       "pfv1", 1)
        b1state["fk1"] = load_f32_half(Kd.ap(), 1, 1, nc.sync.dma_start,
                                       "pfk1", 1)
        b1state["fq1"] = load_f32_half(Qd.ap(), 1, 1, nc.scalar.dma_start,
                                       "pfq1", 1)
        ctx.__exit__(None, None, None)

    def b1_cast(key):
        g = stag.tile([P, HT, D], fp16, name="pg" + key, bufs=1)
        nc.vector.tensor_copy(out=g[:, :, :], in_=b1state["f" + key][:, :, :])
        b1state["g" + key] = g

    def b1_cast_v(h):
        nc.vector.tensor_copy(out=vsb[1][h][:, :, :],
                              in_=b1state[f"fv{h}"][:, :, :])

    b1_phases = {
        (1, 2): b1_loads,
        (1, 5): lambda: b1_cast("k0"),
        (1, 7): lambda: b1_cast("q0"),
        (1, 9): lambda: (b1_cast_v(0), b1_cast_v(1)),
        (1, 11): lambda: transpose_half(b1state["gk0"], ktt[1][0], ktt[1][1]),
        (1, 13): lambda: transpose_half(b1state["gq0"], qt[1][0], qt[1][1]),
        (2, 1): lambda: b1_cast("k1"),
        (2, 3): lambda: transpose_half(b1state["gk1"], ktt[1][2], ktt[1][3]),
        (2, 5): lambda: b1_cast("q1"),
        (2, 7): lambda: transpose_half(b1state["gq1"], qt[1][2], qt[1][3]),
    }

    # ---- deferred epilogue: emitted across the NEXT qc's first k-slots so
    # the in-order PE/DVE streams never head-of-line block on it.
    HQ = NQS // 2

    def make_epilogue(b, qc, acc, ops, pt15, flush=False):
        st = {}

        def stage0():  # denominators (tiny PE matmuls) + reciprocal
            den = tpsum.tile([P, NQS], fp32, name="tps")
            for sq in range(NQS):
                nc.tensor.matmul(den[:, sq:sq + 1],
                                 lhsT=acc[:, sq * P:(sq + 1) * P],
                                 rhs=ones_col[:, :],
                                 start=True, stop=True,
                                 skip_group_check=True)
            rcol = outp.tile([P, NQS], fp32, name="rcol")
            nc.vector.reciprocal(out=rcol[:, :], in_=den[:, :])
            st["rcol"] = rcol

        def stage1():  # PSUM -> SBUF fp16 copies (frees opsum for next qc)
            ots = []
            for hh in range(2):
                ot = outp.tile([P, HQ * P], fp16, name="ot")
                if ACT_COPIES:
                    nc.scalar.copy(
                        out=ot[:, :],
                        in_=ops[:, hh * HQ * P:(hh + 1) * HQ * P])
                else:
                    nc.vector.tensor_copy(
                        out=ot[:, :],
                        in_=ops[:, hh * HQ * P:(hh + 1) * HQ * P])
                ots.append(ot)
            st["ots"] = ots

        def half(hh, split=1):  # transpose + per-subtile scale + store
            ot = st["ots"][hh]
            rcol = st["rcol"]
            QW = HQ // split
            for w in range(split):
                osb = tpsum.tile([P, QW * P], fp16, name="tps")
                for t in range(QW):
                    tt = w * QW + t
                    nc.tensor.transpose(osb[:, t * P:(t + 1) * P],
                                        ot[:, tt * P:(tt + 1) * P],
                                        ident[:, :])
                osf = outp.tile([P, QW, D], fp32, name="osf")
                for t in range(QW):
                    tt = hh * HQ + w * QW + t
                    nc.vector.tensor_scalar_mul(
                        out=osf[:, t, :],
                        in0=osb[:, t * P:(t + 1) * P],
                        scalar1=rcol[:, tt:tt + 1])
                ring = nc.scalar.dma_start if (hh + w) % 2 == 0 \
                    else nc.sync.dma_start
                base = qc * QC + (hh * HQ + w * QW) * P
                ring(out=Od.ap()[b, base:base + QW * P, :]
                     .rearrange("(t p) d -> p t d", p=P),
                     in_=osf[:, :, :])

        return [stage0, stage1, lambda: half(0), lambda: half(1)]

    pending = None

    # ---- main flash loop over (batch, q-chunk, k-tile) ----
    for b in range(BP):
        for qc in range(NQC):
            gi = b * NQC + qc
            qtb, kttb, vsbb = qt[b], ktt[b], vsb[b]
            if gi == 0:
                mf = mf00
            else:
                mf = [load_mask_ck(b, qc, ck) for ck in range(S // MC)]
            acc = accp.tile([P, QC], fp16, name="acc")
            ops = opsum.tile([P, QC], fp32, name="opsum")
            pts = {}
            for kt in range(NKT):
                if pending is not None and kt < 4:
                    pending[kt]()
                    if kt == 3:
                        pending = None
                if BP > 1 and (gi, kt) in b1_phases:
                    b1_phases[(gi, kt)]()
                if gi == 0 and kt == HT - 3:
                    finish_b0_cast()
                if gi == 0 and kt == HT - 1:
                    finish_b0_transpose()
                sc = spsum.tile([P, QC], fp32, name="scores")
                mfck = mf[kt * P // MC]
                kcol = (kt * P) % MC
                kh, kloc = kt // QT4, (kt % QT4) * P

                def emit_masks(first):
                    if MASK_DR:
                        for sq in range(0, NQS, 2):
                            nc.tensor.matmul(
                                sc[:, sq * P:(sq + 2) * P],
                                lhsT=mfck[:, sq:sq + 2, kcol:kcol + P],
                                rhs=negI[:, :, :],
                                start=first and (sq % (MM_N // P) == 0),
                                stop=not first,
                                perf_mode=mybir.MatmulPerfMode.DoubleRow,
                                skip_group_check=True)
                    else:
                        # plain fp8 mask matmuls: on HW, eight 128-col
                        # matmuls with small stationaries pipeline better
                        # than four DoubleRow ones (ldweights dominate);
                        # start=True only on the first matmul per bank
                        for sq in range(NQS):
                            nc.tensor.matmul(
                                sc[:, sq * P:(sq + 1) * P],
                                lhsT=mfck[:, sq, kcol:kcol + P],
                                rhs=negI[:, 0, :P],
                                start=first and (sq % (MM_N // P) == 0),
                                stop=(not first)
                                and (sq % (MM_N // P) == MM_N // P - 1),
                                skip_group_check=True)

                def emit_qk(first):
                    for n in range(0, QC, MM_N):
                        nc.tensor.matmul(
                            sc[:, n:n + MM_N],
                            lhsT=kttb[kh][:, kloc:kloc + P],
                            rhs=qtb[qc * 2 + n // MM_N][:, :],
                            start=first, stop=not first,
                            skip_group_check=True)

                if QK_FIRST:
                    emit_qk(True)
                    emit_masks(False)
                else:
                    emit_masks(True)
                    emit_qk(False)
                pt = pp.tile([P, QC], fp16, name="pt")
                nc.scalar.activation(out=pt[:, :], in_=sc[:, :],
                                     func=Exp, scale=SCALE)
                if kt == 0:
                    nc.vector.tensor_copy(out=acc[:, :], in_=pt[:, :])
                else:
                    nc.vector.tensor_add(out=acc[:, :], in0=acc[:, :],
                                         in1=pt[:, :])
                pts[kt] = pt
                # PV lags TWO k-tiles: the previous qc's stage1 copies have
                # drained opsum before the start=True write lands (no WAR
                # stall), and the PE never waits on exp(kt).
                if kt >= PV_LAG:
                    j = kt - PV_LAG
                    for n in range(0, QC, MM_N):
                        nc.tensor.matmul(
                            ops[:, n:n + MM_N],
                            lhsT=vsbb[j // HT][:, j % HT, :],
                            rhs=pts[j][:, n:n + MM_N],
                            start=(kt == PV_LAG), stop=False,
                            skip_group_check=True)
                    del pts[j]
            for j in range(NKT - PV_LAG, NKT):
                for n in range(0, QC, MM_N):
                    nc.tensor.matmul(
                        ops[:, n:n + MM_N],
                        lhsT=vsbb[j // HT][:, j % HT, :],
                        rhs=pts[j][:, n:n + MM_N],
                        start=False, stop=(j == NKT - 1),
                        skip_group_check=True)
            is_last = (b, qc) == (BP - 1, NQC - 1)
            pending = make_epilogue(b, qc, acc, ops, pts[NKT - 1],
                                    flush=is_last)
    # flush the final qc's epilogue (short tail; overlaps the next
    # iteration's prep DMAs in loop mode)
    for fn in pending:
        fn()


def _get_nc(loop=False):
    key = f"nc_loop{loop}"
    if key not in _CACHE:
        _CACHE[key] = build_nc(loop=loop)
    return _CACHE[key]


def kernel(Q, K, V, mask, dk=128):
    from concourse.bass_utils import run_bass_kernel_spmd

    assert int(dk) == 128
    Q = np.ascontiguousarray(np.asarray(Q, dtype=np.float32))
    K = np.ascontiguousarray(np.asarray(K, dtype=np.float32))
    V = np.ascontiguousarray(np.asarray(V, dtype=np.float32))
    mask_u8 = np.ascontiguousarray(np.asarray(mask)).astype(np.uint8)

    nc = _get_nc(loop=False)
    in_maps = []
    for c in range(NCORES):
        sl = slice(c * BP, (c + 1) * BP)
        in_maps.append({
            "Q": np.ascontiguousarray(Q[sl]),
            "K": np.ascontiguousarray(K[sl]),
            "V": np.ascontiguousarray(V[sl]),
            "mask": np.ascontiguousarray(mask_u8[sl]),
        })
    res = run_bass_kernel_spmd(nc, in_maps, core_ids=list(range(NCORES)))
    return np.concatenate([r["out"] for r in res.results], axis=0)

